# revision 19
# baseline (speedup 1.0000x reference)
"""Trainium2 Bass kernel for nn_GPQSoftMaxNet (vq_codebook).

The reference einsum('nbd,bdc->nc', f, P)/n_book collapses to a plain GEMM:
    out = features @ Prototypes / 16        # [N, D] @ [D, C]
with N=32768, D=256, C=4096, fp32.

Strategy (data-parallel, per sharding hint):
  - shard features rows N across 8 cores (4096 rows each), replicate Prototypes
  - host prep: features shard transposed to featT [D, n_shard] fp16 (so no
    on-chip transpose is needed: the GEMM's stationary operand wants K on
    partitions), Prototypes pre-scaled by 1/16 and cast to fp16
  - per core: fp16 matmul (fp32 PSUM accumulate) on the tensor engine
      * 2 k-tiles of 128 on partitions; per 128-row output strip the k-outer
        loop does LDW(featT[k] strip) once then streams all 8 c-chunks of 512,
        accumulating across the 2 k-tiles in 8 PSUM banks
      * PSUM evacuated with plain copies (cast fp32->fp16) alternating
        Vector/Scalar engines into an fp16 strip, one 1 MB DMA per strip
  - output DRAM tensor is fp16 [n_shard, C]; host concatenates the 8 shards
    and upcasts to fp32

fp16 inputs + fp32 accumulate + fp16 output give ~1e-3 max relative error
vs the fp32 reference (inputs are randn, so no range issues).
"""

import sys

if "/opt/trn_rl_repo" not in sys.path:
    sys.path.insert(0, "/opt/trn_rl_repo")

from contextlib import ExitStack

import numpy as np

import concourse.bass as bass  # noqa: F401
import concourse.mybir as mybir
import concourse.tile as tile
from concourse import bacc
from concourse.bass_utils import run_bass_kernel_spmd

N_CORES = 8
N_FULL = 32768
D = 256
C = 4096
N_SHARD = N_FULL // N_CORES  # 4096

FP16 = mybir.dt.float16
F32 = mybir.dt.float32


def emit(tc, out, featT, protos, repeat=1, tb=4, cb=512, evac="vs",
         loads="scalar", prefetch_s=2, wide_psum=True, out_bufs=3):
    """Emit the per-core kernel body.

    out:    DRAM [n_shard, C] fp16 (ExternalOutput)
    featT:  DRAM [D, n_shard] fp16 (this core's shard, pre-transposed)
    protos: DRAM [D, C] fp16 (replicated, pre-scaled by 1/16)
    """
    nc = tc.nc
    d, n_shard = featT.shape
    _, n_classes = protos.shape
    KT = d // 128          # k-tiles (2)
    NT = n_shard // 128    # output row strips (32)
    CB = cb                # c-chunk per matmul (512 = one PSUM bank of fp32)
    NCH = n_classes // CB
    TB = tb                # strips batched per output DMA

    # out rows (s*TB + b)*128 + p, viewed as [NT//TB, 128, TB, n_classes]
    out_r = out.rearrange("(s b p) c -> s p b c", p=128, b=TB)

    with ExitStack() as ctx:
        # Pools live across repeat iterations so consecutive iterations
        # pipeline (iter i+1's input DMAs overlap iter i's compute).
        PW = 2 * CB if wide_psum else CB   # psum tile width (cols of f32)
        G = PW // CB                       # matmul chunks per psum tile
        in_pool = ctx.enter_context(tc.tile_pool(name="inp", bufs=2))
        mm_psum = ctx.enter_context(
            tc.tile_pool(name="mmps", bufs=8 // (PW // 512), space="PSUM")
        )
        out_pool = ctx.enter_context(tc.tile_pool(name="ostrip", bufs=out_bufs))

        load_eng = {"scalar": nc.scalar, "gpsimd": nc.gpsimd, "sync": nc.sync}[loads]

        def load_inputs():
            # Input loads ride a different DMA ring than the output stores
            # (SP ring) so iteration i+1's loads overlap iteration i's stores.
            # interleaved (f0, p0, f1, p1) so a cold start can begin the
            # first strip's k=0 matmuls after 2 MB instead of 4 MB
            F_sb, P_sb = [], []
            for k in range(KT):
                ft = in_pool.tile([128, n_shard], FP16, tag=f"f{k}", name=f"f_sb{k}")
                load_eng.dma_start(out=ft[:], in_=featT[k * 128:(k + 1) * 128, :])
                F_sb.append(ft)
                pt = in_pool.tile([128, n_classes], FP16, tag=f"p{k}", name=f"p_sb{k}")
                load_eng.dma_start(out=pt[:], in_=protos[k * 128:(k + 1) * 128, :])
                P_sb.append(pt)
            return F_sb, P_sb

        cur = load_inputs()
        for r in range(repeat):
            F_sb, P_sb = cur
            nxt = None

            # --- main loop: out[t*128:(t+1)*128, :] = F[:, strip].T @ P ---
            for s in range(NT // TB):
                if s == prefetch_s and r + 1 < repeat:
                    # software prefetch: next iteration's inputs load while
                    # this iteration computes
                    nxt = load_inputs()
                strip = out_pool.tile(
                    [128, TB, n_classes], FP16, tag="st", name="strip"
                )
                for b in range(TB):
                    t = s * TB + b
                    pss = [
                        mm_psum.tile([128, PW], F32, tag="mm", name=f"ps{j}")
                        for j in range(NCH // G)
                    ]
                    for k in range(KT):
                        w = F_sb[k][:, t * 128:(t + 1) * 128]
                        for ch in range(NCH):
                            nc.tensor.matmul(
                                pss[ch // G][:, (ch % G) * CB:(ch % G + 1) * CB],
                                w,
                                P_sb[k][:, ch * CB:(ch + 1) * CB],
                                start=(k == 0),
                                stop=(k == KT - 1),
                            )
                    for j in range(NCH // G):
                        dst = strip[:, b, j * PW:(j + 1) * PW]
                        if evac == "v" or (evac == "vs" and j % 2 == 0):
                            nc.vector.tensor_copy(dst, pss[j][:])
                        else:
                            nc.scalar.copy(dst, pss[j][:])
                nc.sync.dma_start(out=out_r[s], in_=strip[:])
            cur = nxt if nxt is not None else cur


def _dedup_ldweights(nc):
    """Drop InstLdweights whose weight AP equals the already-loaded one.

    The tile layer splits every matmul into an Ldweights+Matmult pair, so a
    k-outer loop that reuses one stationary tile for 8 matmuls still emits 8
    identical weight loads.  The PE sequencer pays ~160 ns dispatch per
    Ldweights, which makes the NX the critical path (512 of them per
    iteration vs 64 distinct weights).  Weight state only changes at an
    Ldweights (no transposes here), so identical consecutive loads on the PE
    are dead.  Loads carrying sync waits/updates are kept.
    """
    removed = 0
    for blk in nc.m.functions[0].blocks:
        cur = None
        kept = []
        for inst in blk.instructions:
            if type(inst).__name__ == "InstLdweights":
                key = str(inst.ins[0])
                si = inst.sync_info
                has_sync = bool(si and (si.on_wait or si.on_update))
                if key == cur and not has_sync:
                    removed += 1
                    continue
                cur = key
            kept.append(inst)
        blk.instructions = kept
    return removed


def build(n_shard=N_SHARD, n_classes=C, d=D, repeat=1, dedup_ldw=False, **cfg):
    """Build + compile the per-core Bass module."""
    nc = bacc.Bacc(
        "TRN2",
        target_bir_lowering=False,
        debug=False,
        num_devices=N_CORES,
    )
    featT = nc.dram_tensor(
        "featT", [d, n_shard], FP16, kind="ExternalInput"
    ).ap()
    protos = nc.dram_tensor(
        "prototypes", [d, n_classes], FP16, kind="ExternalInput"
    ).ap()
    out = nc.dram_tensor(
        "out", [n_shard, n_classes], FP16, kind="ExternalOutput"
    ).ap()
    with tile.TileContext(nc) as tc:
        emit(tc, out, featT, protos, repeat=repeat, **cfg)
    if dedup_ldw:
        _dedup_ldweights(nc)
    nc.compile()
    return nc


_NC_CACHE = {}


def _get_nc(repeat=1, **cfg):
    key = (repeat, tuple(sorted(cfg.items())))
    if key not in _NC_CACHE:
        _NC_CACHE[key] = build(repeat=repeat, **cfg)
    return _NC_CACHE[key]


def prep_in_maps(features: np.ndarray, Prototypes: np.ndarray):
    """Host-side shard/layout prep shared by kernel() and the test harness."""
    feat16 = np.asarray(features, dtype=np.float16)
    protos16 = (np.asarray(Prototypes, dtype=np.float32) / 16.0).astype(np.float16)
    in_maps = []
    for i in range(N_CORES):
        shard = feat16[i * N_SHARD:(i + 1) * N_SHARD]
        in_maps.append(
            {
                "featT": np.ascontiguousarray(shard.T),
                "prototypes": protos16,
            }
        )
    return in_maps


def kernel(features: np.ndarray, Prototypes: np.ndarray) -> np.ndarray:
    features = np.asarray(features)
    Prototypes = np.asarray(Prototypes)
    assert features.shape == (N_FULL, D), features.shape
    assert Prototypes.shape == (D, C), Prototypes.shape

    nc = _get_nc()
    in_maps = prep_in_maps(features, Prototypes)
    res = run_bass_kernel_spmd(nc, in_maps, list(range(N_CORES)))
    return np.concatenate(
        [res.results[i]["out"] for i in range(N_CORES)], axis=0
    ).astype(np.float32)
